# revision 1
# baseline (speedup 1.0000x reference)
"""Per-sample depthwise 7x7 SAME cross-correlation on 8 trn2 NeuronCores.

Problem: inputs [32,128,128,128] (B,H,W,C), kernels [32,7,7,128] (B,KH,KW,C).
out[b,y,x,c] = sum_{i,j} inputs[b, y+i-3, x+j-3, c] * kernels[b,i,j,c]

Strategy (pure data parallel, batch sharded 4 samples/core):
  - Host: transpose to channel-major [b, c, y, x], zero-pad spatially to
    134x134 (SAME padding built in), convert the image to bf16.
  - On-chip layout: C=128 on partitions, (y, x) in the free dim. Each tap is
    a per-(b,c) scalar multiply of a shifted window, summed over 49 taps.
  - The 49 taps are split across FOUR compute lanes that run concurrently:

    * PE lane (N_PE taps): the TensorEngine multiplies a whole shifted
      window by the per-channel tap scalar as a matmul with a DIAGONAL
      stationary matrix diag(w[b,:,tap]), accumulating all its taps into
      PSUM in f32 for free. The diagonal stationary is built in one DVE
      tensor_scalar op per (sample, tap): IDENT * w (IDENT is a constant
      0/1 identity tile). bf16 moving data streams 1 column/cycle at
      2.4 GHz -> 0.417 ns/elem/tap, ~2.4x any elementwise engine.
    * Act lane: ScalarE computes bf16 products (0.83 ns/elem,
      dtype-agnostic); DVE accumulates them at the bf16 2x rate (0.52).
    * DVE lane: self-contained products at the bf16 4x tensor_scalar rate
      (0.26) + 2x adds (0.52).
    * Pool lane: GpSimd accumulates Act-produced products with its Add
      ucode (1.98 ns/elem); f32 accumulator. (Pool supports no
      tensor_scalar/stt opcodes on TRN2, so it cannot make products.)

  - PSUM: each 16-row slab uses 4 single-bank [128,4,128] f32 tiles
    (matmul moving free dim is capped at 512), double-buffered = 8 banks.
  - Merge on DVE: A0 += A1 (bf16 2x), A0 += P (mixed), then four
    quarter-adds OUT[q] = A0[q] + PSUM[q] producing the f32 output.
  - Accuracy: PE/Pool lanes accumulate in f32; only the 15 Act/DVE-lane
    taps ride bf16 accumulators -> measured ~6.5e-3 max rel err vs the
    f32 reference (harness gate 2e-2).

  Rejected via cost-model sweeps: folding the elementwise accumulators
  into PSUM with IDENT-stationary matmuls (slab-boundary serialization
  loses to the parallel DVE merge), Pool-side merge adds (cross-engine
  semaphore hops), and fused scalar_tensor_tensor MACs (no DVE fast
  modes). Pool cannot run tensor_scalar/stt at all on TRN2 (ISA check).
"""

import numpy as np
import ml_dtypes

import concourse.bass as bass
import concourse.tile as tile
from concourse import bacc, mybir
from concourse.bass_utils import run_bass_kernel_spmd

B, H, W, C = 32, 128, 128, 128
KH = KW = 7
PAD = 3
N_CORES = 8
BPC = B // N_CORES  # samples per core
HP, WP = H + 2 * PAD, W + 2 * PAD  # 134, 134
SLAB = 16  # output rows per compute slab (fits 4 psum banks)
N_SLABS = H // SLAB
QROWS = 4  # rows per matmul: 4*128 = 512 = max moving free size

# Tap split across the lanes (balanced via the instruction cost model).
N_PE = 29    # diag-matmul taps on the TensorEngine
N_ACT = 7    # ScalarE products accumulated by DVE
N_DVE = 8    # product+add pairs fully on DVE
N_POOL = 49 - N_PE - N_ACT - N_DVE  # Act products accumulated by GpSimd
POOL_CHAINS = 1

_PROGRAM_CACHE = {}


def _build_program(repeat=1, n_pe=N_PE, n_act=N_ACT, n_dve=N_DVE, slab=SLAB,
                   pool_chains=POOL_CHAINS, q_adds_on="dve", merge="dve",
                   evac="act"):
    f32 = mybir.dt.float32
    bf16 = mybir.dt.bfloat16
    n_pool = 49 - n_pe - n_act - n_dve
    assert n_act >= 2 and n_pool >= pool_chains and n_pe >= 1
    taps = [(i, j) for i in range(KH) for j in range(KW)]
    pe_taps = taps[:n_pe]
    a_taps = taps[n_pe : n_pe + n_act]
    d_taps = taps[n_pe + n_act : n_pe + n_act + n_dve]
    g_taps = taps[n_pe + n_act + n_dve :]
    n_slabs = H // slab
    nq = slab // QROWS
    mult, add = mybir.AluOpType.mult, mybir.AluOpType.add

    nc = bacc.Bacc("TRN2", target_bir_lowering=False, debug=False)
    x_h = nc.dram_tensor("x", [BPC, C, HP, WP], bf16, kind="ExternalInput")
    w_h = nc.dram_tensor("w", [BPC, C, KH * KW], f32, kind="ExternalInput")
    o_h = nc.dram_tensor("o", [BPC, C, H, W], f32, kind="ExternalOutput")
    x, w, o = x_h.ap(), w_h.ap(), o_h.ap()

    with tile.TileContext(nc) as tc:
        with (
            tc.tile_pool(name="wpool", bufs=1) as wpool,
            tc.tile_pool(name="dpool", bufs=2) as dpool,
            tc.tile_pool(name="xpool", bufs=4) as xpool,
            tc.tile_pool(name="a0p", bufs=3) as a0p,
            tc.tile_pool(name="a1p", bufs=3) as a1p,
            tc.tile_pool(name="pcp", bufs=3) as pcp,
            tc.tile_pool(name="tmpa", bufs=6) as tmpap,
            tc.tile_pool(name="tmpd", bufs=3) as tmpdp,
            tc.tile_pool(name="outp", bufs=3) as outp,
            tc.psum_pool(name="ps", bufs=2) as ps,
        ):
            ones = wpool.tile([C, 128], bf16)
            ident = wpool.tile([C, 128], bf16)
            nc.vector.memset(ones, 1.0)
            nc.gpsimd.affine_select(
                out=ident, in_=ones, pattern=[[1, 128]],
                compare_op=mybir.AluOpType.is_equal, fill=0.0,
                base=0, channel_multiplier=-1,
            )
            wall = wpool.tile([C, BPC, KH * KW], f32)
            for b in range(BPC):
                nc.sync.dma_start(out=wall[:, b, :], in_=w[b])

            def emit_finish(pending):
                """Deferred slab finish: PE folds accs into PSUM, evac, DMA."""
                pt, accs, fb, fy0 = pending
                for k, acc in enumerate(accs):
                    for q in range(nq):
                        nc.tensor.matmul(
                            out=pt[q][:, :, :],
                            lhsT=ident,
                            rhs=acc[:, QROWS * q : QROWS * (q + 1), :],
                            start=False,
                            stop=(k == len(accs) - 1),
                        )
                out_t = outp.tile([C, slab, W], f32, name="out_t")
                for q in range(nq):
                    dst = out_t[:, QROWS * q : QROWS * (q + 1), :]
                    if evac == "act":
                        nc.scalar.copy(dst, pt[q])
                    else:
                        nc.vector.tensor_copy(dst, pt[q])
                nc.sync.dma_start(out=o[fb, :, fy0 : fy0 + slab, :], in_=out_t)

            pending = None
            last_b = None
            for b, s in [
                (b, s)
                for _ in range(repeat)
                for b in range(BPC)
                for s in range(n_slabs)
            ]:
                if b != last_b:
                    # Per-sample diagonal stationaries for the PE taps.
                    diag = dpool.tile([C, n_pe, 128], bf16, name="diag")
                    for t, (i, j) in enumerate(pe_taps):
                        nc.vector.tensor_scalar_mul(
                            diag[:, t, :], ident,
                            wall[:, b, i * KW + j : i * KW + j + 1],
                        )
                    last_b = b

                y0 = s * slab
                xt = xpool.tile([C, slab + 2 * PAD, WP], bf16)
                nc.sync.dma_start(out=xt, in_=x[b, :, y0 : y0 + slab + 2 * PAD, :])

                def xin(t, r0=0, r1=slab):
                    i, j = t
                    return xt[:, i + r0 : i + r1, j : j + W]

                def wsc(t):
                    i, j = t
                    return wall[:, b, t[0] * KW + t[1] : t[0] * KW + t[1] + 1]

                # --- PE lane: diag matmuls accumulating into PSUM --------
                pt = [
                    ps.tile([C, QROWS, W], f32, name=f"pt{q}", tag=f"pt{q}")
                    for q in range(nq)
                ]
                # Tap-outer order: the 4 bank-matmuls of a tap share one
                # stationary load (29 ldweights/slab, each hidden behind a
                # 213 ns matmul). Bank-outer staggering of the accumulation
                # groups simmed only 0.16% better while quadrupling the
                # ldweights churn the cost model does not price — not worth
                # the hardware risk.
                for t, tap in enumerate(pe_taps):
                    for q in range(nq):
                        nc.tensor.matmul(
                            out=pt[q][:, :, :],
                            lhsT=diag[:, t, :],
                            rhs=xin(tap, QROWS * q, QROWS * (q + 1)),
                            start=(t == 0),
                            stop=(merge == "dve" and t == len(pe_taps) - 1),
                        )

                # --- Act lane: bf16 products (2 seed the DVE accs) -------
                aacc = [
                    a0p.tile([C, slab, W], bf16, name="a0", tag="a0"),
                    a1p.tile([C, slab, W], bf16, name="a1", tag="a1"),
                ]
                nc.scalar.mul(aacc[0], xin(a_taps[0]), wsc(a_taps[0]))
                nc.scalar.mul(aacc[1], xin(a_taps[1]), wsc(a_taps[1]))
                # Pool accumulators seeded directly by Act.
                pdt = f32 if merge == "dve" else bf16
                pacc = [
                    pcp.tile([C, slab, W], pdt, name=f"pc{k}", tag=f"pc{k}")
                    for k in range(pool_chains)
                ]
                for k in range(pool_chains):
                    nc.scalar.mul(pacc[k], xin(g_taps[k]), wsc(g_taps[k]))
                # Remaining products, interleaved so Pool is fed steadily.
                a_rest = [("a", t) for t in a_taps[2:]]
                g_rest = [("g", t) for t in g_taps[pool_chains:]]
                prods = []
                na, ng = len(a_rest), len(g_rest)
                ia = ig = 0
                for k in range(na + ng):
                    if ig < ng and (ia >= na or ig * (na + ng) <= k * ng):
                        prods.append(g_rest[ig]); ig += 1
                    else:
                        prods.append(a_rest[ia]); ia += 1
                act_out = []
                for kind, tap in prods:
                    tmp = tmpap.tile([C, slab, W], bf16, name="atmp")
                    nc.scalar.mul(tmp, xin(tap), wsc(tap))
                    act_out.append((kind, tmp))

                # --- Pool lane: accumulate its products ------------------
                gch = 0
                for kind, tmp in act_out:
                    if kind == "g":
                        nc.gpsimd.tensor_add(pacc[gch], pacc[gch], tmp)
                        gch = (gch + 1) % pool_chains

                # --- DVE lane: Act-product adds + own pairs --------------
                dve_stream = [("act", tmp) for kind, tmp in act_out
                              if kind == "a"]
                step = max(1, (len(dve_stream) + len(d_taps)) // max(1, len(d_taps)))
                for k, tap in enumerate(d_taps):
                    pos = min(len(dve_stream), (k + 1) * step - 1)
                    dve_stream.insert(pos, ("dve", tap))
                ch = 0
                for kind, payload in dve_stream:
                    if kind == "act":
                        nc.vector.tensor_add(aacc[ch], aacc[ch], payload)
                    else:
                        tmp = tmpdp.tile([C, slab, W], bf16, name="dtmp")
                        nc.vector.tensor_scalar_mul(tmp, xin(payload), wsc(payload))
                        nc.vector.tensor_add(aacc[ch], aacc[ch], tmp)
                    ch ^= 1

                # --- Merge + output --------------------------------------
                if merge == "dvepool":
                    # Pool absorbs the A-chain fold (it idles at slab end);
                    # DVE only does the bf16 chain fold + psum quarter-adds.
                    out_t = outp.tile([C, slab, W], f32, name="out_t")
                    nc.vector.tensor_add(aacc[0], aacc[0], aacc[1])
                    for k in range(1, pool_chains):
                        nc.gpsimd.tensor_add(pacc[0], pacc[0], pacc[k])
                    nc.gpsimd.tensor_add(pacc[0], pacc[0], aacc[0])
                    for q in range(nq):
                        nc.vector.tensor_add(
                            out_t[:, QROWS * q : QROWS * (q + 1), :],
                            pacc[0][:, QROWS * q : QROWS * (q + 1), :],
                            pt[q],
                        )
                    nc.sync.dma_start(out=o[b, :, y0 : y0 + slab, :], in_=out_t)
                elif merge == "dve":
                    out_t = outp.tile([C, slab, W], f32, name="out_t")
                    nc.vector.tensor_add(aacc[0], aacc[0], aacc[1])
                    for k in range(1, pool_chains):
                        nc.vector.tensor_add(pacc[0], pacc[0], pacc[k])
                    nc.vector.tensor_add(aacc[0], aacc[0], pacc[0])
                    for q in range(nq):
                        if q_adds_on == "dve":
                            q_eng = nc.vector
                        elif q_adds_on == "pool":
                            q_eng = nc.gpsimd
                        else:  # "split": Pool takes half, it idles at slab end
                            q_eng = nc.gpsimd if q < nq // 2 else nc.vector
                        q_eng.tensor_add(
                            out_t[:, QROWS * q : QROWS * (q + 1), :],
                            aacc[0][:, QROWS * q : QROWS * (q + 1), :],
                            pt[q],
                        )
                    nc.sync.dma_start(out=o[b, :, y0 : y0 + slab, :], in_=out_t)
                else:
                    emit_finish((pt, aacc + pacc, b, y0))

    nc.compile()
    return nc


def _get_program():
    if "nc" not in _PROGRAM_CACHE:
        _PROGRAM_CACHE["nc"] = _build_program()
    return _PROGRAM_CACHE["nc"]


def _prep_inputs(inputs, kernels):
    """Host-side shard + layout transform. Returns per-core input maps."""
    xt = _PROGRAM_CACHE.get("xt")
    if xt is None:
        xt = np.zeros((B, C, HP, WP), ml_dtypes.bfloat16)
        _PROGRAM_CACHE["xt"] = xt
    xt[:, :, PAD : PAD + H, PAD : PAD + W] = np.transpose(
        inputs, (0, 3, 1, 2)
    ).astype(ml_dtypes.bfloat16)
    wt = np.ascontiguousarray(
        np.transpose(kernels, (0, 3, 1, 2)).reshape(B, C, KH * KW)
    )
    in_maps = []
    for k in range(N_CORES):
        sl = slice(k * BPC, (k + 1) * BPC)
        in_maps.append({"x": xt[sl], "w": wt[sl]})
    return in_maps


def _gather_output(results):
    full = np.concatenate([r["o"] for r in results], axis=0)  # [B, C, H, W]
    return np.ascontiguousarray(np.transpose(full, (0, 2, 3, 1)))


def run_spmd(inputs, kernels, **spmd_kwargs):
    """Run on all 8 cores; returns (output, BassKernelResults)."""
    nc = _get_program()
    in_maps = _prep_inputs(np.asarray(inputs), np.asarray(kernels))
    res = run_bass_kernel_spmd(nc, in_maps, list(range(N_CORES)), **spmd_kwargs)
    return _gather_output(res.results), res


def kernel(inputs, kernels):
    out, _ = run_spmd(inputs, kernels)
    return out



# revision 3
# speedup vs baseline: 3.0294x; 3.0294x over previous
"""Per-sample depthwise 7x7 SAME cross-correlation on 8 trn2 NeuronCores.

Problem: inputs [32,128,128,128] (B,H,W,C), kernels [32,7,7,C].
out[b,h,w,c] = sum_{i,j} inputs[b, h+i-3, w+j-3, c] * kernels[b,i,j,c]

Strategy (pure data parallel, batch sharded 4 samples/core), v2:
  All 49 taps run on the TensorEngine as BANDED-TOEPLITZ matmuls --
  ~7 MACs per moving column instead of the 1 a diagonal stationary
  gives, so the PE does the whole conv at ~7x its diag rate and the
  elementwise engines only evacuate PSUM.

  - Partition space p = (ci, ws): 4 channels x 32 width positions.
    The moving tile per (b, cgroup-of-4) is [128, 5 wblk, 134 hpad]
    bf16: five OVERLAPPING width blocks (stride 26, halo 3 each side,
    zero-padded outside the image) so no cross-block seams exist;
    only block outputs ws in [3, 29) are valid and the host discards
    the rest. H padding (3+128+3) lives in the free dim: kernel row i
    is a free-dim slice [i : i+128] -- shifts are free.
  - Stationary per (b, i, cgroup): S[p=(ci,ws), o=(co,wo)] =
    (ci==co) * w[b, i, ws-wo+3, c] -- block-diagonal of four 32x32
    7-banded Toeplitz blocks. Engines cannot write per-partition
    -offset diagonals at rate, so the 896 stationaries per core are
    PREBUILT BY THE HOST and shipped as an extra 28.7 MB input --
    DMA is the one resource with headroom (total in+S+out ~72 MB at
    ~330 GB/s ~ 215 us, balanced against ~245 us of PE).
  - Per (b, cg): 7 accumulation matmuls into PSUM P0 [128,4,128] f32
    (one full bank, N=512) + 7 into P1 [128,128] (N=128), start/stop
    framing the i-group. ldweights (128 cols, FWL-eligible) hides
    behind the 213 ns N=512 matmuls.
  - Act evacuates PSUM to bf16 SBUF (f32 accumulation throughout; only
    the final store rounds to bf16: measured ~4e-3 max rel err vs the
    f32 reference, gate 2e-2), DMA out 21 MB bf16; host casts to f32
    and reassembles valid slices.

  Rejected: on-chip Toeplitz construction (no engine can write
  per-partition-offset diagonals faster than ~1 full-tile pass per
  stationary; gather/scatter live on GpSimd at ~2 ns/elem), Winograd
  (bf16 transform conditioning blows the 2e-2 gate), FFT (freq dims
  need >128 partitions), elementwise tap lanes (engine-sum floor
  ~650 us -- the v1 kernel at 890 us was already near it).
"""

import numpy as np
import ml_dtypes

import concourse.bass as bass
import concourse.tile as tile
from concourse import bacc, mybir
from concourse.bass_utils import run_bass_kernel_spmd

B, H, W, C = 32, 128, 128, 128
KH = KW = 7
PAD = 3
N_CORES = 8
BPC = B // N_CORES          # samples per core
HP = H + 2 * PAD            # 134 padded rows (free dim)
CSUB = 4                    # channels per partition tile
WSUB = 32                   # width positions per channel in partitions
NCG = C // CSUB             # 32 channel groups
VAL = WSUB - 2 * PAD        # 26 valid outputs per width block
NBLK = -(-W // VAL)         # 5 overlapping width blocks (stride VAL)

_PROGRAM_CACHE = {}


def _build_program():
    f32 = mybir.dt.float32
    bf16 = mybir.dt.bfloat16

    nc = bacc.Bacc("TRN2", target_bir_lowering=False, debug=False)
    x_h = nc.dram_tensor("x", [BPC, NCG, 128, NBLK, HP], bf16,
                         kind="ExternalInput")
    s_h = nc.dram_tensor("s", [BPC, NCG, 128, KH, 128], bf16,
                         kind="ExternalInput")
    o_h = nc.dram_tensor("o", [BPC, NCG, 128, NBLK, H], bf16,
                         kind="ExternalOutput")
    x, s, o = x_h.ap(), s_h.ap(), o_h.ap()

    with tile.TileContext(nc) as tc:
        with (
            tc.tile_pool(name="xpool", bufs=3) as xpool,
            tc.tile_pool(name="spool", bufs=3) as spool,
            tc.tile_pool(name="outp", bufs=3) as outp,
            tc.psum_pool(name="ps", bufs=2) as ps,
        ):
            for b in range(BPC):
                for g in range(NCG):
                    xt = xpool.tile([128, NBLK, HP], bf16, name="xt")
                    st = spool.tile([128, KH, 128], bf16, name="st")
                    nc.sync.dma_start(out=xt, in_=x[b, g])
                    nc.sync.dma_start(out=st, in_=s[b, g])

                    p0 = ps.tile([128, NBLK - 1, H], f32, name="p0", tag="p0")
                    p1 = ps.tile([128, H], f32, name="p1", tag="p1")
                    for i in range(KH):
                        nc.tensor.matmul(
                            out=p0,
                            lhsT=st[:, i, :],
                            rhs=xt[:, : NBLK - 1, i : i + H],
                            start=(i == 0),
                            stop=(i == KH - 1),
                        )
                        nc.tensor.matmul(
                            out=p1,
                            lhsT=st[:, i, :],
                            rhs=xt[:, NBLK - 1, i : i + H],
                            start=(i == 0),
                            stop=(i == KH - 1),
                        )

                    ot = outp.tile([128, NBLK, H], bf16, name="ot")
                    nc.scalar.copy(ot[:, : NBLK - 1, :], p0)
                    nc.scalar.copy(ot[:, NBLK - 1, :], p1)
                    nc.sync.dma_start(out=o[b, g], in_=ot)

    nc.compile()
    return nc


def _get_program():
    if "nc" not in _PROGRAM_CACHE:
        _PROGRAM_CACHE["nc"] = _build_program()
    return _PROGRAM_CACHE["nc"]


def _prep_inputs(inputs, kernels):
    """Host-side shard + layout transform. Returns per-core input maps."""
    bf16 = ml_dtypes.bfloat16
    # ---- moving tiles: [B, NCG, (ci, ws), NBLK, HP] ----------------------
    # padded image in [B, C, W, H] with W padded to cover block 4's halo
    # (w in [-3, VAL*NBLK + PAD) -> offset +3, width WPAD) and H padded +-3.
    WPAD = VAL * NBLK + 2 * PAD  # 136 >= last block start 101 + WSUB
    xp = np.zeros((B, C, WPAD, HP), dtype=bf16)
    xp[:, :, PAD : PAD + W, PAD : PAD + H] = np.transpose(
        inputs, (0, 3, 2, 1)
    ).astype(bf16)
    # blocks: block k covers padded-W slice [k*VAL, k*VAL + WSUB)
    st = xp.strides
    blocks = np.lib.stride_tricks.as_strided(
        xp,
        shape=(B, C, NBLK, WSUB, HP),
        strides=(st[0], st[1], st[2] * VAL, st[2], st[3]),
    )
    # -> [B, cg, ci, ws, blk, HP] -> [B, NCG, 128, NBLK, HP]
    xt = np.ascontiguousarray(
        blocks.reshape(B, NCG, CSUB, NBLK, WSUB, HP).transpose(0, 1, 2, 4, 3, 5)
    ).reshape(B, NCG, 128, NBLK, HP)

    # ---- stationaries: [B, NCG, p=(ci,ws), i, o=(co,wo)] -----------------
    # S[p, i, o] = (ci==co) * w[b, i, ws-wo+3, c];  j = ws-wo+3 in [0, 7)
    ws = np.arange(WSUB)[:, None]
    wo = np.arange(WSUB)[None, :]
    j = ws - wo + PAD                      # [WSUB, WSUB]
    valid = (j >= 0) & (j < KW)
    jc = np.clip(j, 0, KW - 1)
    # kernels [B, KH, KW, C] -> kt [B, C, KH, KW]
    kt = np.transpose(np.asarray(kernels), (0, 3, 1, 2))
    # bands [B, C, KH, WSUB, WSUB] = kt[b, c, i, jc[ws, wo]] * valid
    bands = (kt[:, :, :, jc] * valid).astype(bf16)
    S = np.zeros((B, NCG, CSUB, WSUB, KH, CSUB, WSUB), dtype=bf16)
    ii = np.arange(CSUB)
    # place each channel's band on the (ci == co) diagonal
    S[:, :, ii, :, :, ii] = (
        bands.reshape(B, NCG, CSUB, KH, WSUB, WSUB)
        .transpose(2, 0, 1, 4, 3, 5)[ii]
    )
    S = np.ascontiguousarray(S).reshape(B, NCG, 128, KH, 128)

    in_maps = []
    for k in range(N_CORES):
        sl = slice(k * BPC, (k + 1) * BPC)
        in_maps.append({"x": xt[sl], "s": S[sl]})
    return in_maps


def _gather_output(results):
    # o [BPC, NCG, (ci, wo), NBLK, H] bf16 per core
    full = np.concatenate([r["o"] for r in results], axis=0).reshape(
        B, NCG, CSUB, WSUB, NBLK, H
    )
    # valid outputs: block k, wo in [PAD, PAD+VAL) -> w = k*VAL + wo - PAD
    out = np.empty((B, NCG, CSUB, W, H), dtype=np.float32)
    for k in range(NBLK):
        n = min(VAL, W - k * VAL)
        out[:, :, :, k * VAL : k * VAL + n, :] = full[
            :, :, :, PAD : PAD + n, k, :
        ].astype(np.float32)
    # [B, cg, ci, W, H] -> [B, H, W, C]
    return np.ascontiguousarray(
        out.transpose(0, 4, 3, 1, 2).reshape(B, H, W, C)
    )


def run_spmd(inputs, kernels, **spmd_kwargs):
    """Run on all 8 cores; returns (output, BassKernelResults)."""
    nc = _get_program()
    in_maps = _prep_inputs(np.asarray(inputs), np.asarray(kernels))
    res = run_bass_kernel_spmd(nc, in_maps, list(range(N_CORES)), **spmd_kwargs)
    return _gather_output(res.results), res


def kernel(inputs, kernels):
    out, _ = run_spmd(inputs, kernels)
    return out
